# revision 9
# baseline (speedup 1.0000x reference)
"""Per-image piecewise-linear LUT (histogram binning) kernel for Trainium2.

v2 strategy (pure data-parallel over 8 NeuronCores, batch sharded 2 per core):
- Host converts x to fp16 (clamped to 1-2^-11 so indices stay < 512) and builds
  a 512-bin fine-grained table per (b, c): node values of the reference PWL at
  j/512, packed per bin as (y0*1024 fp16 | dy*1024 fp16 << 16) in one u32.
  The x1024 scale keeps tiny outputs out of the fp16-subnormal range; the host
  divides it back out after the run.
- On-device per core, per chunk of [128 x 4096] fp16 pixels:
    u16  = u16(512*x - 0.5)          (DVE tensor_scalar, 4x mode; fp32->u16
                                      convert is round-nearest => floor)
    s16  = 512*x                     (ACT engine, exact: pow2 scale)
    e    = pooltable[u16]            (Pool engine PoolBufferLoad+Gather,
                                      512-entry per-image table, u32 out)
    fr   = s16 - u16                 (DVE tensor_tensor, 2x; in-place over s16)
    a    = lo16(e) dense             (ACT copy from stride-2 fp16 view)
    d    = hi16(e) dense             (DVE copy, 2x)
    m    = fr * d                    (DVE, 2x)
    o    = m + a                     (DVE, 2x, fp16 out scaled by 1024)
- The gather runs at a fixed ~3.54 ns per element-column and is the bottleneck
  (~175us/core); DVE (~125us), ACT (~90us) and DMA (~70us) hide under it.
- Raw Gather/PoolBufferLoad ISA instructions cannot carry semaphores; drains
  bracket them and cross-engine syncs land on the drains (wired manually).
"""

import sys

sys.path.insert(0, "/opt/trn_rl_repo")

import numpy as np

B, C, H, W = 16, 3, 1024, 1024
K = 64
NCORES = 8
BPC = B // NCORES  # batches per core
IMGS = BPC * C  # images per core
P = 128
FREE = H * W // P  # 8192
CHUNK = 4096
NCHUNK = FREE // CHUNK
NB = 2  # buffer slots
TBL = 512  # fine bins (pool buffer entries)
SC = 1024.0  # table value scale (host divides back out)

_cached = {}


def _get_fracmul_op():
    """m = (s - (((s - 0.5) + M) - M)) * in1 with s = in0*s0, M = 1.5*2^23.
    The fp32 RNE magic turns the inner term into max(rint(s-0.5), 0) == the
    u16 gather index, so this computes frac(s)*dy in ONE DVE pass."""
    from concourse import dve_ops
    from concourse.dve_spec import Spec, Src0, Src1, C0, C1, C2, lower
    from concourse.dve_uop import DveOpSpec

    for op in dve_ops.OPS:
        if op.name == "ANT_FRACMUL_MAGIC":
            return op
    s_ = Src0 * C0
    body = (s_ - (((s_ - C1) + C2) - C2)) * Src1
    spec = Spec(
        body=body,
        reference=lambda in0, in1, s0, s1, imm2: (
            (lambda s: (s - np.maximum(np.rint(s - s1), 0.0)) * in1)(
                np.asarray(in0, dtype=np.float32) * np.float32(s0)
            )
        ),
    )
    opcode = dve_ops._CUSTOM_DVE_ROW_BASE + len(dve_ops.OPS)
    sha = {}
    for ver in ("v3", "v4"):
        s = DveOpSpec(
            name="ANT_FRACMUL_MAGIC",
            opcode=opcode,
            uops=lower(spec, ver=ver),
            rd1_en=True,
        )
        sha[ver] = s.sha(ver)
    op = dve_ops.DveOp("ANT_FRACMUL_MAGIC", spec, subdim=False, uops_sha=sha)
    dve_ops.OPS.append(op)
    dve_ops._SUB_OPCODE_FOR_NAME[op.name] = opcode
    dve_ops.CUSTOM_DVE_SPECS[op.name] = spec
    return op


def _get_idx_op():
    """u16 = convert(in0*s0 - s1), round-nearest => floor index.
    Single-source custom DVE op (rd0 only): the stock tensor_scalar runs in
    4x mode which occupies both shared ports and stalls against the gather."""
    from concourse import dve_ops
    from concourse.dve_spec import Spec, Src0, C0, C1, lower
    from concourse.dve_uop import DveOpSpec

    for op in dve_ops.OPS:
        if op.name == "ANT_IDX_SCALE":
            return op
    spec = Spec(
        body=Src0 * C0 - C1,
        reference=lambda in0, s0, s1, imm2: in0 * s0 - s1,
    )
    opcode = dve_ops._CUSTOM_DVE_ROW_BASE + len(dve_ops.OPS)
    sha = {}
    for ver in ("v3", "v4"):
        s = DveOpSpec(
            name="ANT_IDX_SCALE",
            opcode=opcode,
            uops=lower(spec, ver=ver),
            rd1_en=False,
        )
        sha[ver] = s.sha(ver)
    op = dve_ops.DveOp("ANT_IDX_SCALE", spec, subdim=False, uops_sha=sha)
    dve_ops.OPS.append(op)
    dve_ops._SUB_OPCODE_FOR_NAME[op.name] = opcode
    dve_ops.CUSTOM_DVE_SPECS[op.name] = spec
    return op


def _build():
    import concourse.mybir as mybir
    from concourse.bacc import Bacc
    from concourse.tile import TileContext
    from concourse.tile_rust import add_dep_helper
    import concourse.bass_interp as _bi

    # Tile's scheduling simulator doesn't know these opcodes; no-op them there.
    _orig_visit = _bi._visit_InstISA

    def _patched_visit(isa, instruction, core_sim):
        if instruction.isa_opcode in (
            isa.Opcode.NEURON_ISA_TPB_OPCODE_POOL_BUFFER_LOAD.value,
            isa.Opcode.NEURON_ISA_TPB_OPCODE_GATHER.value,
        ):
            return
        return _orig_visit(isa, instruction, core_sim)

    _bi._visit_InstISA = _patched_visit

    fm_op = _get_fracmul_op()

    nc = Bacc()
    dt = nc.isa.get_enum("NEURON_ISA_TPB_DTYPE")
    Op = nc.isa.Opcode
    ALU = mybir.AluOpType
    U16 = dt.NEURON_ISA_TPB_DTYPE_UINT16.value
    U32 = dt.NEURON_ISA_TPB_DTYPE_UINT32.value

    xs_d = nc.dram_tensor("xs", [IMGS, H, W], mybir.dt.float16, kind="ExternalInput")
    tb_d = nc.dram_tensor("tb", [IMGS, P, TBL], mybir.dt.uint32, kind="ExternalInput")
    os_d = nc.dram_tensor("os", [IMGS, H, W], mybir.dt.float16, kind="ExternalOutput")

    xs_r = xs_d[:].rearrange("i (p r) c -> i p (r c)", p=P)
    os_r = os_d[:].rearrange("i (p r) c -> i p (r c)", p=P)

    with (
        nc.sbuf_tensor("tbl_all", [P, IMGS * TBL], mybir.dt.uint32) as tbl_all,
        nc.sbuf_tensor("xb", [P, NB * CHUNK], mybir.dt.float16) as xb,
        nc.sbuf_tensor("ub", [P, NB * CHUNK], mybir.dt.uint16) as ub,
        nc.sbuf_tensor("eb", [P, NB * CHUNK], mybir.dt.uint32) as eb,
        nc.sbuf_tensor("ab", [P, NB * CHUNK], mybir.dt.float16) as ab,
        nc.sbuf_tensor("db", [P, NB * CHUNK], mybir.dt.float16) as db,
        nc.sbuf_tensor("mb", [P, NB * CHUNK], mybir.dt.float16) as mb,
        nc.sbuf_tensor("ob", [P, NB * CHUNK], mybir.dt.float16) as ob,
        TileContext(nc) as tc,
    ):
        ub_off, _ = nc.gpsimd._ap_to_byte_offset(ub[:])
        eb_off, _ = nc.gpsimd._ap_to_byte_offset(eb[:])
        tbl_off, _ = nc.gpsimd._ap_to_byte_offset(tbl_all[:])

        for img in range(IMGS):
            nc.sync.dma_start(tbl_all[:, img * TBL : (img + 1) * TBL], tb_d[img])
        tbl_touch = nc.vector.tensor_copy(
            eb[:, : IMGS * TBL], tbl_all[:]
        )  # pool waits collapse onto the DVE clock

        e16 = eb[:].bitcast(mybir.dt.float16).rearrange("p (n two) -> p n two", two=2)

        prev_gather = None
        prev_pre = None
        pend = None  # previous chunk, awaiting its post-gather fence
        unpack_hist = {}  # slot -> (unpack_a, unpack_d) last readers of eb slot

        def _emit_interp(p):
            """Interp for chunk p, fenced on p['post'] (its gather is done)."""
            so = p["so"]
            a_t = ab[:, so : so + CHUNK]
            d_t = db[:, so : so + CHUNK]
            # unpack the gathered (y0, dy) pair into dense fp16 arrays
            up_a = nc.scalar.copy(a_t, e16[:, so : so + CHUNK, 0])
            add_dep_helper(up_a.ins, p["post"].ins, sync=True, reason="g done")
            up_d = nc.scalar.copy(d_t, e16[:, so : so + CHUNK, 1])
            add_dep_helper(up_d.ins, p["post"].ins, sync=True, reason="g done")
            m_t = mb[:, so : so + CHUNK]
            o_t = ob[:, so : so + CHUNK]
            nc.vector._custom_dve(
                fm_op, out=m_t, in0=p["x_t"], in1=d_t,
                s0=512.0, s1=0.5, imm2=1.5 * 2.0**23,
            )
            nc.vector.tensor_tensor(o_t, m_t, a_t, ALU.add)
            nc.sync.dma_start(os_r[p["img"], :, p["f0"] : p["f0"] + CHUNK], o_t)
            unpack_hist[p["slot"]] = (up_a, up_d)

        k = 0
        for img in range(IMGS):
            for cidx in range(NCHUNK):
                slot = k % NB
                f0 = cidx * CHUNK
                so = slot * CHUNK
                x_t = xb[:, so : so + CHUNK]
                u_t = ub[:, so : so + CHUNK]

                nc.sync.dma_start(x_t, xs_r[img, :, f0 : f0 + CHUNK])

                # u = u16(512x - 0.5) on the ACT engine: convert-on-write is
                # round-nearest, matching the DVE magic in the fracmul op.
                # Fence on pre[k-1] (ordered after gather[k-2]) so this write
                # can't race the still-running gather reading ub's old slot.
                ts_u = nc.scalar.activation(
                    u_t, x_t, mybir.ActivationFunctionType.Copy, bias=-0.5, scale=512.0
                )
                if prev_pre is not None:
                    add_dep_helper(ts_u.ins, prev_pre.ins, sync=True, reason="ub WAR")

                # pool: single drain per chunk — previous gather's completion
                # fence AND this gather's input wait
                pre = nc.gpsimd.drain()
                if prev_gather is not None:
                    add_dep_helper(
                        pre.ins, prev_gather.ins, sync=False, reason="pool order"
                    )
                add_dep_helper(pre.ins, ts_u.ins, sync=True, reason="u ready")
                if k >= NB:
                    for up in unpack_hist[slot]:
                        add_dep_helper(pre.ins, up.ins, sync=True, reason="e WAR")
                if k == 0:
                    add_dep_helper(pre.ins, tbl_touch.ins, sync=True, reason="tables")

                gdep = pre
                if cidx == 0:
                    pbl = nc.gpsimd.isa(
                        Op.NEURON_ISA_TPB_OPCODE_POOL_BUFFER_LOAD,
                        {
                            "src_mem_pattern": {
                                "start_addr": {
                                    "addr_immediate": int(tbl_off) + img * TBL * 4
                                },
                                "num_elem": [TBL, 1, 1, 1],
                                "step_elem": [1, 0, 0, 0],
                            },
                            "in_dtype": U32,
                            "num_active_channels": P,
                            "start_index": 0,
                            "mask": TBL - 1,
                        },
                    )
                    add_dep_helper(pbl.ins, pre.ins, sync=False, reason="pool order")
                    gdep = pbl
                gt = nc.gpsimd.isa(
                    Op.NEURON_ISA_TPB_OPCODE_GATHER,
                    {
                        "src_mem_pattern": {
                            "start_addr": {"addr_immediate": int(ub_off) + so * 2},
                            "num_elem": [CHUNK, 1, 1, 1],
                            "step_elem": [1, 0, 0, 0],
                        },
                        "dst_mem_pattern": {
                            "start_addr": {"addr_immediate": int(eb_off) + so * 4},
                            "num_elem": [CHUNK, 1, 1, 1],
                            "step_elem": [1, 0, 0, 0],
                        },
                        "in_dtype": U16,
                        "out_dtype": U32,
                        "num_active_channels": P,
                        "index_miss_behavior": 0,
                        "immediate": {"imm_bitvec_uint32": 0},
                        "free_pool_buffer": 0,
                    },
                )
                add_dep_helper(gt.ins, gdep.ins, sync=False, reason="pool order")

                # interp of the PREVIOUS chunk fences on pre (its gather done)
                if pend is not None:
                    pend["post"] = pre
                    _emit_interp(pend)
                pend = dict(slot=slot, so=so, x_t=x_t, img=img, f0=f0, post=None)
                prev_gather = gt
                prev_pre = pre
                k += 1

        fin = nc.gpsimd.drain()
        add_dep_helper(fin.ins, prev_gather.ins, sync=False, reason="pool order")
        pend["post"] = fin
        _emit_interp(pend)

    nc.finalize()
    return nc


def _tables(un_normalized_y: np.ndarray) -> np.ndarray:
    """[B, C, TBL] u32 packed (fp16 SC*y0[j] | fp16 SC*dy[j] << 16), 512 bins."""
    u = un_normalized_y.astype(np.float64)
    h = np.logaddexp(0.0, u)  # softplus
    y = np.cumsum(h, axis=2)
    y0 = y[:, :, :1]
    yn = y[:, :, -1:]
    y = (y - y0) / (yn - y0)  # [B, C, K+1] exact nodes, y[0]=0, y[K]=1

    # fine node values f(j/TBL), j=0..TBL (exact: linear within coarse bins)
    t = np.arange(TBL + 1) / TBL
    s = t * K
    j = np.clip(np.floor(s), 0, K - 1).astype(int)
    fr = s - j
    a = np.take_along_axis(y, np.broadcast_to(j, (B, C, TBL + 1)), axis=2)
    b = np.take_along_axis(y, np.broadcast_to(j + 1, (B, C, TBL + 1)), axis=2)
    fnode = a + fr * (b - a)  # [B, C, TBL+1]

    a16 = (fnode[:, :, :TBL] * SC).astype(np.float16).view(np.uint16).astype(np.uint32)
    d16 = (
        ((fnode[:, :, 1:] - fnode[:, :, :TBL]) * SC)
        .astype(np.float16)
        .view(np.uint16)
        .astype(np.uint32)
    )
    return a16 | (d16 << 16)  # [B, C, TBL]


def _in_maps(x: np.ndarray, uy: np.ndarray):
    pk = _tables(uy)
    x16 = np.minimum(x.astype(np.float16), np.float16(1.0 - 2.0**-11))
    in_maps = []
    for c in range(NCORES):
        xs = x16[c * BPC : (c + 1) * BPC].reshape(IMGS, H, W)
        tb = np.ascontiguousarray(
            np.broadcast_to(
                pk[c * BPC : (c + 1) * BPC].reshape(IMGS, 1, TBL), (IMGS, P, TBL)
            )
        )
        in_maps.append({"xs": np.ascontiguousarray(xs), "tb": tb})
    return in_maps


def _assemble(res) -> np.ndarray:
    out = np.empty((B, C, H, W), dtype=np.float32)
    for c in range(NCORES):
        out[c * BPC : (c + 1) * BPC] = (
            res.results[c]["os"].astype(np.float32).reshape(BPC, C, H, W)
        )
    out *= 1.0 / SC
    return out


def kernel(x: np.ndarray, un_normalized_y: np.ndarray) -> np.ndarray:
    from concourse import bass_utils

    x = np.asarray(x, dtype=np.float32)
    uy = np.asarray(un_normalized_y, dtype=np.float32)

    if "nc" not in _cached:
        _cached["nc"] = _build()
    nc = _cached["nc"]

    res = bass_utils.run_bass_kernel_spmd(
        nc, _in_maps(x, uy), core_ids=list(range(NCORES))
    )
    return _assemble(res)


# revision 10
# speedup vs baseline: 1.5189x; 1.5189x over previous
"""Per-image piecewise-linear LUT (histogram binning) kernel for Trainium2.

Strategy (pure data-parallel over 8 NeuronCores, batch sharded 2 per core):
- Host converts x to fp16 (clamped to 1-2^-11 so indices stay < 512) and builds
  a 512-bin fine-grained table per (b, c): node values of the reference PWL at
  j/512, packed per bin as (y0*1024 fp16 | dy*1024 fp16 << 16) in one u32.
  The x1024 scale keeps tiny outputs out of the fp16-subnormal range; the host
  divides it back out after the run.
- Per chunk of [128 x 4096] fp16 pixels, work is spread so that the Pool
  engine's fixed-rate gather (~3.54 ns/elem-column, the bottleneck) shares its
  SBUF port with as little DVE traffic as possible:
    ACT   u16 = cvt(512x - 0.5)      (convert-on-write is round-nearest)
    DVE   fr  = s - rne(s - 0.5)     (single-src custom op via fp32 magic
                                      rounding; rd0-only, s = 512x)
    POOL  e   = pooltable[u16]       (PoolBufferLoad + Gather, u32 out)
    ACT   a   = lo16(e), d = hi16(e) (strided->dense unpacks, full rate)
    DVE   m   = fr * d               (tensor_tensor, 2x packed fp16)
    DVE   o   = m + a                (tensor_tensor, 2x, scaled fp16 out)
- Three-stage software pipeline (index/frac one chunk ahead of the gather,
  interp one chunk behind) keeps the pool queue from waiting on ACT/DVE.
- Raw Gather/PoolBufferLoad ISA instructions cannot carry semaphores; drains
  bracket them and cross-engine syncs land on the drains (wired manually).
"""

import sys

sys.path.insert(0, "/opt/trn_rl_repo")

import numpy as np

B, C, H, W = 16, 3, 1024, 1024
K = 64
NCORES = 8
BPC = B // NCORES  # batches per core
IMGS = BPC * C  # images per core
P = 128
FREE = H * W // P  # 8192
CHUNK = 4096
NCHUNK = FREE // CHUNK
TOTAL = IMGS * NCHUNK
NBX = 3  # x/u/fr buffer slots (stage-A lead)
NB = 2  # e/a/d/m/o buffer slots
TBL = 512  # fine bins (pool buffer entries)
SC = 1024.0  # table value scale (host divides back out)

_cached = {}


def _get_frac_op():
    """fr = s - (((s - 0.5) + M) - M) with s = in0*s0, M = 1.5*2^23 (fp32 RNE
    magic => fr = s - max(rint(s-0.5), 0), exactly matching the u16 index).
    Single-source custom DVE op: uses only the unshared rd0 port."""
    from concourse import dve_ops
    from concourse.dve_spec import Spec, Src0, C0, C1, C2, lower
    from concourse.dve_uop import DveOpSpec

    for op in dve_ops.OPS:
        if op.name == "ANT_FRAC_MAGIC":
            return op
    s_ = Src0 * C0
    body = s_ - (((s_ - C1) + C2) - C2)
    spec = Spec(
        body=body,
        reference=lambda in0, s0, s1, imm2: (
            (lambda s: s - np.maximum(np.rint(s - s1), 0.0))(
                np.asarray(in0, dtype=np.float32) * np.float32(s0)
            )
        ),
    )
    opcode = dve_ops._CUSTOM_DVE_ROW_BASE + len(dve_ops.OPS)
    sha = {}
    for ver in ("v3", "v4"):
        s = DveOpSpec(
            name="ANT_FRAC_MAGIC",
            opcode=opcode,
            uops=lower(spec, ver=ver),
            rd1_en=False,
        )
        sha[ver] = s.sha(ver)
    op = dve_ops.DveOp("ANT_FRAC_MAGIC", spec, subdim=False, uops_sha=sha)
    dve_ops.OPS.append(op)
    dve_ops._SUB_OPCODE_FOR_NAME[op.name] = opcode
    dve_ops.CUSTOM_DVE_SPECS[op.name] = spec
    return op


def _build():
    import concourse.mybir as mybir
    from concourse.bacc import Bacc
    from concourse.tile import TileContext
    from concourse.tile_rust import add_dep_helper
    import concourse.bass_interp as _bi

    # Tile's scheduling simulator doesn't know these opcodes; no-op them there.
    _orig_visit = _bi._visit_InstISA

    def _patched_visit(isa, instruction, core_sim):
        if instruction.isa_opcode in (
            isa.Opcode.NEURON_ISA_TPB_OPCODE_POOL_BUFFER_LOAD.value,
            isa.Opcode.NEURON_ISA_TPB_OPCODE_GATHER.value,
        ):
            return
        return _orig_visit(isa, instruction, core_sim)

    _bi._visit_InstISA = _patched_visit

    frac_op = _get_frac_op()

    nc = Bacc()
    dt = nc.isa.get_enum("NEURON_ISA_TPB_DTYPE")
    Op = nc.isa.Opcode
    ALU = mybir.AluOpType
    U16 = dt.NEURON_ISA_TPB_DTYPE_UINT16.value
    U32 = dt.NEURON_ISA_TPB_DTYPE_UINT32.value

    xs_d = nc.dram_tensor("xs", [IMGS, H, W], mybir.dt.float16, kind="ExternalInput")
    tb_d = nc.dram_tensor("tb", [IMGS, P, TBL], mybir.dt.uint32, kind="ExternalInput")
    os_d = nc.dram_tensor("os", [IMGS, H, W], mybir.dt.float16, kind="ExternalOutput")

    xs_r = xs_d[:].rearrange("i (p r) c -> i p (r c)", p=P)
    os_r = os_d[:].rearrange("i (p r) c -> i p (r c)", p=P)

    with (
        nc.sbuf_tensor("tbl_all", [P, IMGS * TBL], mybir.dt.uint32) as tbl_all,
        nc.sbuf_tensor("xb", [P, NBX * CHUNK], mybir.dt.float16) as xb,
        nc.sbuf_tensor("ub", [P, NBX * CHUNK], mybir.dt.uint16) as ub,
        nc.sbuf_tensor("fb", [P, NBX * CHUNK], mybir.dt.float16) as fb,
        nc.sbuf_tensor("eb", [P, NB * CHUNK], mybir.dt.uint32) as eb,
        nc.sbuf_tensor("ab", [P, NB * CHUNK], mybir.dt.float16) as ab,
        nc.sbuf_tensor("db", [P, NB * CHUNK], mybir.dt.float16) as db,
        nc.sbuf_tensor("mb", [P, NB * CHUNK], mybir.dt.float16) as mb,
        nc.sbuf_tensor("ob", [P, NB * CHUNK], mybir.dt.float16) as ob,
        TileContext(nc) as tc,
    ):
        ub_off, _ = nc.gpsimd._ap_to_byte_offset(ub[:])
        eb_off, _ = nc.gpsimd._ap_to_byte_offset(eb[:])
        tbl_off, _ = nc.gpsimd._ap_to_byte_offset(tbl_all[:])

        for img in range(IMGS):
            nc.sync.dma_start(tbl_all[:, img * TBL : (img + 1) * TBL], tb_d[img])
        tbl_touch = nc.vector.tensor_copy(
            eb[:, : IMGS * TBL], tbl_all[:]
        )  # pool waits collapse onto the DVE clock

        e16 = eb[:].bitcast(mybir.dt.float16).rearrange("p (n two) -> p n two", two=2)

        idx_ins = {}  # k -> ACT index op
        pre_ins = {}  # k -> pool pre-drain (retires => gather[k-1] done)
        up_ins = {}  # k -> (up_a, up_d)
        gather_ins = {}  # k -> gather

        def stage_a(k):
            """DMA in + index (ACT) + frac (DVE), one chunk ahead."""
            img, cidx = divmod(k, NCHUNK)
            sx = (k % NBX) * CHUNK
            x_t = xb[:, sx : sx + CHUNK]
            u_t = ub[:, sx : sx + CHUNK]
            f_t = fb[:, sx : sx + CHUNK]
            nc.sync.dma_start(x_t, xs_r[img, :, cidx * CHUNK : (cidx + 1) * CHUNK])
            # u16 = cvt(512x - 0.5); ACT convert-on-write is round-nearest.
            ts_u = nc.scalar.activation(
                u_t, x_t, mybir.ActivationFunctionType.Copy, bias=-0.5, scale=512.0
            )
            if k >= NBX:
                # ub-slot WAR: gather[k-NBX] read this slot; pre[k-NBX+1]
                # retiring guarantees it finished.
                add_dep_helper(
                    ts_u.ins, pre_ins[k - NBX + 1].ins, sync=True, reason="ub WAR"
                )
            idx_ins[k] = ts_u
            # fr = 512x - rne(512x - 0.5): rd0-only custom op. fb-slot WAR is
            # tile-tracked (m[k-NBX] is a stock reader of fb).
            nc.vector._custom_dve(
                frac_op, out=f_t, in0=x_t, s0=512.0, s1=0.5, imm2=1.5 * 2.0**23
            )

        def stage_b(k):
            """Pool: pre-drain, per-image PBL, gather."""
            img, cidx = divmod(k, NCHUNK)
            sx = (k % NBX) * CHUNK
            se = (k % NB) * CHUNK
            pre = nc.gpsimd.drain()
            if k > 0:
                add_dep_helper(
                    pre.ins, gather_ins[k - 1].ins, sync=False, reason="pool order"
                )
            add_dep_helper(pre.ins, idx_ins[k].ins, sync=True, reason="u ready")
            if k >= NB:
                for up in up_ins[k - NB]:
                    add_dep_helper(pre.ins, up.ins, sync=True, reason="e WAR")
            if k == 0:
                add_dep_helper(pre.ins, tbl_touch.ins, sync=True, reason="tables")
            pre_ins[k] = pre
            gdep = pre
            if cidx == 0:
                pbl = nc.gpsimd.isa(
                    Op.NEURON_ISA_TPB_OPCODE_POOL_BUFFER_LOAD,
                    {
                        "src_mem_pattern": {
                            "start_addr": {
                                "addr_immediate": int(tbl_off) + img * TBL * 4
                            },
                            "num_elem": [TBL, 1, 1, 1],
                            "step_elem": [1, 0, 0, 0],
                        },
                        "in_dtype": U32,
                        "num_active_channels": P,
                        "start_index": 0,
                        "mask": TBL - 1,
                    },
                )
                add_dep_helper(pbl.ins, pre.ins, sync=False, reason="pool order")
                gdep = pbl
            gt = nc.gpsimd.isa(
                Op.NEURON_ISA_TPB_OPCODE_GATHER,
                {
                    "src_mem_pattern": {
                        "start_addr": {"addr_immediate": int(ub_off) + sx * 2},
                        "num_elem": [CHUNK, 1, 1, 1],
                        "step_elem": [1, 0, 0, 0],
                    },
                    "dst_mem_pattern": {
                        "start_addr": {"addr_immediate": int(eb_off) + se * 4},
                        "num_elem": [CHUNK, 1, 1, 1],
                        "step_elem": [1, 0, 0, 0],
                    },
                    "in_dtype": U16,
                    "out_dtype": U32,
                    "num_active_channels": P,
                    "index_miss_behavior": 0,
                    "immediate": {"imm_bitvec_uint32": 0},
                    "free_pool_buffer": 0,
                },
            )
            add_dep_helper(gt.ins, gdep.ins, sync=False, reason="pool order")
            gather_ins[k] = gt

        def stage_c(k, post):
            """Unpack (ACT) + interp (DVE) + DMA out; `post` fences gather[k]."""
            img, cidx = divmod(k, NCHUNK)
            sx = (k % NBX) * CHUNK
            se = (k % NB) * CHUNK
            a_t = ab[:, se : se + CHUNK]
            d_t = db[:, se : se + CHUNK]
            m_t = mb[:, se : se + CHUNK]
            o_t = ob[:, se : se + CHUNK]
            up_a = nc.scalar.copy(a_t, e16[:, se : se + CHUNK, 0])
            add_dep_helper(up_a.ins, post.ins, sync=True, reason="g done")
            up_d = nc.scalar.copy(d_t, e16[:, se : se + CHUNK, 1])
            add_dep_helper(up_d.ins, post.ins, sync=True, reason="g done")
            up_ins[k] = (up_a, up_d)
            nc.vector.tensor_tensor(m_t, fb[:, sx : sx + CHUNK], d_t, ALU.mult)
            nc.vector.tensor_tensor(o_t, m_t, a_t, ALU.add)
            nc.sync.dma_start(
                os_r[img, :, cidx * CHUNK : (cidx + 1) * CHUNK], o_t
            )

        for k in range(TOTAL + 2):
            if k < TOTAL:
                stage_a(k)
            if 0 <= k - 1 < TOTAL:
                stage_b(k - 1)
            if 2 <= k <= TOTAL:
                # gather[k-2] is fenced by pre[k-1] (emitted just above)
                stage_c(k - 2, pre_ins[k - 1])
        fin = nc.gpsimd.drain()
        add_dep_helper(fin.ins, gather_ins[TOTAL - 1].ins, sync=False, reason="order")
        stage_c(TOTAL - 1, fin)

    nc.finalize()
    return nc


def _tables(un_normalized_y: np.ndarray) -> np.ndarray:
    """[B, C, TBL] u32 packed (fp16 SC*y0[j] | fp16 SC*dy[j] << 16), 512 bins."""
    u = un_normalized_y.astype(np.float64)
    h = np.logaddexp(0.0, u)  # softplus
    y = np.cumsum(h, axis=2)
    y0 = y[:, :, :1]
    yn = y[:, :, -1:]
    y = (y - y0) / (yn - y0)  # [B, C, K+1] exact nodes, y[0]=0, y[K]=1

    # fine node values f(j/TBL), j=0..TBL (exact: linear within coarse bins)
    t = np.arange(TBL + 1) / TBL
    s = t * K
    j = np.clip(np.floor(s), 0, K - 1).astype(int)
    fr = s - j
    a = np.take_along_axis(y, np.broadcast_to(j, (B, C, TBL + 1)), axis=2)
    b = np.take_along_axis(y, np.broadcast_to(j + 1, (B, C, TBL + 1)), axis=2)
    fnode = a + fr * (b - a)  # [B, C, TBL+1]

    a16 = (fnode[:, :, :TBL] * SC).astype(np.float16).view(np.uint16).astype(np.uint32)
    d16 = (
        ((fnode[:, :, 1:] - fnode[:, :, :TBL]) * SC)
        .astype(np.float16)
        .view(np.uint16)
        .astype(np.uint32)
    )
    return a16 | (d16 << 16)  # [B, C, TBL]


def _in_maps(x: np.ndarray, uy: np.ndarray):
    pk = _tables(uy)
    x16 = np.minimum(x.astype(np.float16), np.float16(1.0 - 2.0**-11))
    in_maps = []
    for c in range(NCORES):
        xs = x16[c * BPC : (c + 1) * BPC].reshape(IMGS, H, W)
        tb = np.ascontiguousarray(
            np.broadcast_to(
                pk[c * BPC : (c + 1) * BPC].reshape(IMGS, 1, TBL), (IMGS, P, TBL)
            )
        )
        in_maps.append({"xs": np.ascontiguousarray(xs), "tb": tb})
    return in_maps


def _assemble(res) -> np.ndarray:
    out = np.empty((B, C, H, W), dtype=np.float32)
    for c in range(NCORES):
        out[c * BPC : (c + 1) * BPC] = (
            res.results[c]["os"].astype(np.float32).reshape(BPC, C, H, W)
        )
    out *= 1.0 / SC
    return out


def kernel(x: np.ndarray, un_normalized_y: np.ndarray) -> np.ndarray:
    from concourse import bass_utils

    x = np.asarray(x, dtype=np.float32)
    uy = np.asarray(un_normalized_y, dtype=np.float32)

    if "nc" not in _cached:
        _cached["nc"] = _build()
    nc = _cached["nc"]

    res = bass_utils.run_bass_kernel_spmd(
        nc, _in_maps(x, uy), core_ids=list(range(NCORES))
    )
    return _assemble(res)


# revision 12
# speedup vs baseline: 1.9340x; 1.2733x over previous
"""Per-image piecewise-linear LUT (histogram binning) kernel for Trainium2.

Strategy (pure data-parallel over 8 NeuronCores, batch sharded 2 per core):
- Host converts x to fp16 (clamped to 1-2^-11 so indices stay < 512) and builds
  a 512-bin fine-grained table per (b, c): node values of the reference PWL at
  j/512, packed per bin as (y0*1024 fp16 | dy*1024 fp16 << 16) in one u32.
  The x1024 scale keeps tiny outputs out of the fp16-subnormal range; the host
  divides it back out after the run.
- Per chunk of [128 x 4096] fp16 pixels, work is spread so that the Pool
  engine's fixed-rate gather (~3.54 ns/elem-column, the bottleneck) shares its
  SBUF port with as little DVE traffic as possible:
    ACT   u16 = cvt(512x - 0.5)      (convert-on-write is round-nearest)
    DVE   fr  = s - rne(s - 0.5)     (single-src custom op via fp32 magic
                                      rounding; rd0-only, s = 512x)
    POOL  e   = pooltable[u16]       (PoolBufferLoad + Gather, u32 out)
    ACT   a   = lo16(e), d = hi16(e) (strided->dense unpacks, full rate)
    DVE   m   = fr * d               (tensor_tensor, 2x packed fp16)
    DVE   o   = m + a                (tensor_tensor, 2x, scaled fp16 out)
- Three-stage software pipeline (index/frac one chunk ahead of the gather,
  interp one chunk behind) keeps the pool queue from waiting on ACT/DVE.
- Raw Gather/PoolBufferLoad ISA instructions cannot carry semaphores; drains
  bracket them and cross-engine syncs land on the drains (wired manually).
"""

import sys

sys.path.insert(0, "/opt/trn_rl_repo")

import numpy as np

B, C, H, W = 16, 3, 1024, 1024
K = 64
NCORES = 8
BPC = B // NCORES  # batches per core
IMGS = BPC * C  # images per core
P = 128
FREE = H * W // P  # 8192
CHUNK = 2048
NCHUNK = FREE // CHUNK
TOTAL = IMGS * NCHUNK
NBX = 3  # x/u/fr buffer slots (stage-A lead)
NB = 2  # e/a/d/m/o buffer slots
TBL = 512  # fine bins (pool buffer entries)
SC = 1024.0  # table value scale (host divides back out)

_cached = {}


def _get_frac_op():
    """fr = s - (((s - 0.5) + M) - M) with s = in0*s0, M = 1.5*2^23 (fp32 RNE
    magic => fr = s - max(rint(s-0.5), 0), exactly matching the u16 index).
    Single-source custom DVE op: uses only the unshared rd0 port."""
    from concourse import dve_ops
    from concourse.dve_spec import Spec, Src0, C0, C1, C2, lower
    from concourse.dve_uop import DveOpSpec

    for op in dve_ops.OPS:
        if op.name == "ANT_FRAC_MAGIC":
            return op
    s_ = Src0 * C0
    body = s_ - (((s_ - C1) + C2) - C2)
    spec = Spec(
        body=body,
        reference=lambda in0, s0, s1, imm2: (
            (lambda s: s - np.maximum(np.rint(s - s1), 0.0))(
                np.asarray(in0, dtype=np.float32) * np.float32(s0)
            )
        ),
    )
    opcode = dve_ops._CUSTOM_DVE_ROW_BASE + len(dve_ops.OPS)
    sha = {}
    for ver in ("v3", "v4"):
        s = DveOpSpec(
            name="ANT_FRAC_MAGIC",
            opcode=opcode,
            uops=lower(spec, ver=ver),
            rd1_en=False,
        )
        sha[ver] = s.sha(ver)
    op = dve_ops.DveOp("ANT_FRAC_MAGIC", spec, subdim=False, uops_sha=sha)
    dve_ops.OPS.append(op)
    dve_ops._SUB_OPCODE_FOR_NAME[op.name] = opcode
    dve_ops.CUSTOM_DVE_SPECS[op.name] = spec
    return op


def _build():
    import concourse.mybir as mybir
    from concourse.bacc import Bacc
    from concourse.tile import TileContext
    from concourse.tile_rust import add_dep_helper
    import concourse.bass_interp as _bi

    # Tile's scheduling simulator doesn't know these opcodes; no-op them there.
    _orig_visit = _bi._visit_InstISA

    def _patched_visit(isa, instruction, core_sim):
        if instruction.isa_opcode in (
            isa.Opcode.NEURON_ISA_TPB_OPCODE_POOL_BUFFER_LOAD.value,
            isa.Opcode.NEURON_ISA_TPB_OPCODE_GATHER.value,
        ):
            return
        return _orig_visit(isa, instruction, core_sim)

    _bi._visit_InstISA = _patched_visit

    frac_op = _get_frac_op()

    nc = Bacc()
    dt = nc.isa.get_enum("NEURON_ISA_TPB_DTYPE")
    Op = nc.isa.Opcode
    ALU = mybir.AluOpType
    U16 = dt.NEURON_ISA_TPB_DTYPE_UINT16.value
    U32 = dt.NEURON_ISA_TPB_DTYPE_UINT32.value

    xs_d = nc.dram_tensor("xs", [IMGS, H, W], mybir.dt.float16, kind="ExternalInput")
    tb_d = nc.dram_tensor("tb", [IMGS, P, TBL], mybir.dt.uint32, kind="ExternalInput")
    os_d = nc.dram_tensor("os", [IMGS, H, W], mybir.dt.float16, kind="ExternalOutput")

    xs_r = xs_d[:].rearrange("i (p r) c -> i p (r c)", p=P)
    os_r = os_d[:].rearrange("i (p r) c -> i p (r c)", p=P)

    with (
        nc.sbuf_tensor("tbl_all", [P, IMGS * TBL], mybir.dt.uint32) as tbl_all,
        nc.sbuf_tensor("xb", [P, NBX * CHUNK], mybir.dt.float16) as xb,
        nc.sbuf_tensor("ub", [P, NBX * CHUNK], mybir.dt.uint16) as ub,
        nc.sbuf_tensor("fb", [P, NBX * CHUNK], mybir.dt.float16) as fb,
        nc.sbuf_tensor("eb", [P, NB * CHUNK], mybir.dt.uint32) as eb,
        nc.sbuf_tensor("mb", [P, NB * CHUNK], mybir.dt.float16) as mb,
        nc.sbuf_tensor("ob", [P, NB * CHUNK], mybir.dt.float16) as ob,
        nc.psum_tensor("pa", [P, CHUNK], mybir.dt.float32) as pa,
        nc.psum_tensor("pd", [P, CHUNK], mybir.dt.float32) as pd,
        TileContext(nc) as tc,
    ):
        ub_off, _ = nc.gpsimd._ap_to_byte_offset(ub[:])
        eb_off, _ = nc.gpsimd._ap_to_byte_offset(eb[:])
        tbl_off, _ = nc.gpsimd._ap_to_byte_offset(tbl_all[:])

        for img in range(IMGS):
            nc.sync.dma_start(tbl_all[:, img * TBL : (img + 1) * TBL], tb_d[img])
        tbl_touch = nc.vector.tensor_copy(
            eb[:, : IMGS * TBL], tbl_all[:]
        )  # pool waits collapse onto the DVE clock

        e16 = eb[:].bitcast(mybir.dt.float16).rearrange("p (n two) -> p n two", two=2)

        idx_ins = {}  # k -> ACT index op
        pre_ins = {}  # k -> pool pre-drain (retires => gather[k-1] done)
        up_ins = {}  # k -> (up_a, up_d)
        gather_ins = {}  # k -> gather

        def stage_a(k):
            """DMA in + index (ACT) + frac (DVE), one chunk ahead."""
            img, cidx = divmod(k, NCHUNK)
            sx = (k % NBX) * CHUNK
            x_t = xb[:, sx : sx + CHUNK]
            u_t = ub[:, sx : sx + CHUNK]
            f_t = fb[:, sx : sx + CHUNK]
            nc.sync.dma_start(x_t, xs_r[img, :, cidx * CHUNK : (cidx + 1) * CHUNK])
            # u16 = cvt(512x - 0.5); ACT convert-on-write is round-nearest.
            ts_u = nc.scalar.activation(
                u_t, x_t, mybir.ActivationFunctionType.Copy, bias=-0.5, scale=512.0
            )
            if k >= NBX:
                # ub-slot WAR: gather[k-NBX] read this slot; pre[k-NBX+1]
                # retiring guarantees it finished.
                add_dep_helper(
                    ts_u.ins, pre_ins[k - NBX + 1].ins, sync=True, reason="ub WAR"
                )
            idx_ins[k] = ts_u
            # fr = 512x - rne(512x - 0.5): rd0-only custom op. fb-slot WAR is
            # tile-tracked (m[k-NBX] is a stock reader of fb).
            nc.vector._custom_dve(
                frac_op, out=f_t, in0=x_t, s0=512.0, s1=0.5, imm2=1.5 * 2.0**23
            )

        def stage_b(k):
            """Pool: pre-drain, per-image PBL, gather."""
            img, cidx = divmod(k, NCHUNK)
            sx = (k % NBX) * CHUNK
            se = (k % NB) * CHUNK
            pre = nc.gpsimd.drain()
            if k > 0:
                add_dep_helper(
                    pre.ins, gather_ins[k - 1].ins, sync=False, reason="pool order"
                )
            add_dep_helper(pre.ins, idx_ins[k].ins, sync=True, reason="u ready")
            if k >= NB:
                for up in up_ins[k - NB]:
                    add_dep_helper(pre.ins, up.ins, sync=True, reason="e WAR")
            if k == 0:
                add_dep_helper(pre.ins, tbl_touch.ins, sync=True, reason="tables")
            pre_ins[k] = pre
            gdep = pre
            if cidx == 0:
                pbl = nc.gpsimd.isa(
                    Op.NEURON_ISA_TPB_OPCODE_POOL_BUFFER_LOAD,
                    {
                        "src_mem_pattern": {
                            "start_addr": {
                                "addr_immediate": int(tbl_off) + img * TBL * 4
                            },
                            "num_elem": [TBL, 1, 1, 1],
                            "step_elem": [1, 0, 0, 0],
                        },
                        "in_dtype": U32,
                        "num_active_channels": P,
                        "start_index": 0,
                        "mask": TBL - 1,
                    },
                )
                add_dep_helper(pbl.ins, pre.ins, sync=False, reason="pool order")
                gdep = pbl
            gt = nc.gpsimd.isa(
                Op.NEURON_ISA_TPB_OPCODE_GATHER,
                {
                    "src_mem_pattern": {
                        "start_addr": {"addr_immediate": int(ub_off) + sx * 2},
                        "num_elem": [CHUNK, 1, 1, 1],
                        "step_elem": [1, 0, 0, 0],
                    },
                    "dst_mem_pattern": {
                        "start_addr": {"addr_immediate": int(eb_off) + se * 4},
                        "num_elem": [CHUNK, 1, 1, 1],
                        "step_elem": [1, 0, 0, 0],
                    },
                    "in_dtype": U16,
                    "out_dtype": U32,
                    "num_active_channels": P,
                    "index_miss_behavior": 0,
                    "immediate": {"imm_bitvec_uint32": 0},
                    "free_pool_buffer": 0,
                },
            )
            add_dep_helper(gt.ins, gdep.ins, sync=False, reason="pool order")
            gather_ins[k] = gt

        def stage_c(k, post):
            """Unpack (ACT) + interp (DVE) + DMA out; `post` fences gather[k]."""
            img, cidx = divmod(k, NCHUNK)
            sx = (k % NBX) * CHUNK
            se = (k % NB) * CHUNK
            m_t = mb[:, se : se + CHUNK]
            o_t = ob[:, se : se + CHUNK]
            # unpack into fp32 PSUM (single slot; the a/d WAR on the previous
            # chunk's o/m readers is tile-tracked): the PSUM read port is
            # separate from the SBUF port the gather shares with DVE rd1, so
            # the TTs below run without pool contention (1x mode).
            up_a = nc.scalar.copy(pa[:], e16[:, se : se + CHUNK, 0])
            add_dep_helper(up_a.ins, post.ins, sync=True, reason="g done")
            up_d = nc.scalar.copy(pd[:], e16[:, se : se + CHUNK, 1])
            add_dep_helper(up_d.ins, post.ins, sync=True, reason="g done")
            up_ins[k] = (up_a, up_d)
            nc.vector.tensor_tensor(m_t, fb[:, sx : sx + CHUNK], pd[:], ALU.mult)
            nc.vector.tensor_tensor(o_t, m_t, pa[:], ALU.add)
            nc.sync.dma_start(
                os_r[img, :, cidx * CHUNK : (cidx + 1) * CHUNK], o_t
            )

        for k in range(TOTAL + 2):
            if k < TOTAL:
                stage_a(k)
            if 0 <= k - 1 < TOTAL:
                stage_b(k - 1)
            if 2 <= k <= TOTAL:
                # gather[k-2] is fenced by pre[k-1] (emitted just above)
                stage_c(k - 2, pre_ins[k - 1])
        fin = nc.gpsimd.drain()
        add_dep_helper(fin.ins, gather_ins[TOTAL - 1].ins, sync=False, reason="order")
        stage_c(TOTAL - 1, fin)

    nc.finalize()
    return nc


def _tables(un_normalized_y: np.ndarray) -> np.ndarray:
    """[B, C, TBL] u32 packed (fp16 SC*y0[j] | fp16 SC*dy[j] << 16), 512 bins."""
    u = un_normalized_y.astype(np.float64)
    h = np.logaddexp(0.0, u)  # softplus
    y = np.cumsum(h, axis=2)
    y0 = y[:, :, :1]
    yn = y[:, :, -1:]
    y = (y - y0) / (yn - y0)  # [B, C, K+1] exact nodes, y[0]=0, y[K]=1

    # fine node values f(j/TBL), j=0..TBL (exact: linear within coarse bins)
    t = np.arange(TBL + 1) / TBL
    s = t * K
    j = np.clip(np.floor(s), 0, K - 1).astype(int)
    fr = s - j
    a = np.take_along_axis(y, np.broadcast_to(j, (B, C, TBL + 1)), axis=2)
    b = np.take_along_axis(y, np.broadcast_to(j + 1, (B, C, TBL + 1)), axis=2)
    fnode = a + fr * (b - a)  # [B, C, TBL+1]

    a16 = (fnode[:, :, :TBL] * SC).astype(np.float16).view(np.uint16).astype(np.uint32)
    d16 = (
        ((fnode[:, :, 1:] - fnode[:, :, :TBL]) * SC)
        .astype(np.float16)
        .view(np.uint16)
        .astype(np.uint32)
    )
    return a16 | (d16 << 16)  # [B, C, TBL]


def _in_maps(x: np.ndarray, uy: np.ndarray):
    pk = _tables(uy)
    x16 = np.minimum(x.astype(np.float16), np.float16(1.0 - 2.0**-11))
    in_maps = []
    for c in range(NCORES):
        xs = x16[c * BPC : (c + 1) * BPC].reshape(IMGS, H, W)
        tb = np.ascontiguousarray(
            np.broadcast_to(
                pk[c * BPC : (c + 1) * BPC].reshape(IMGS, 1, TBL), (IMGS, P, TBL)
            )
        )
        in_maps.append({"xs": np.ascontiguousarray(xs), "tb": tb})
    return in_maps


def _assemble(res) -> np.ndarray:
    out = np.empty((B, C, H, W), dtype=np.float32)
    for c in range(NCORES):
        out[c * BPC : (c + 1) * BPC] = (
            res.results[c]["os"].astype(np.float32).reshape(BPC, C, H, W)
        )
    out *= 1.0 / SC
    return out


def kernel(x: np.ndarray, un_normalized_y: np.ndarray) -> np.ndarray:
    from concourse import bass_utils

    x = np.asarray(x, dtype=np.float32)
    uy = np.asarray(un_normalized_y, dtype=np.float32)

    if "nc" not in _cached:
        _cached["nc"] = _build()
    nc = _cached["nc"]

    res = bass_utils.run_bass_kernel_spmd(
        nc, _in_maps(x, uy), core_ids=list(range(NCORES))
    )
    return _assemble(res)
